# revision 1
# baseline (speedup 1.0000x reference)
"""CANINE self-attention (relative_key_query) Trainium2 Bass kernel, 8-core SPMD.

Sharding: data-parallel over batch (4) x tensor-parallel over heads (12 -> 2
groups of 6). Core c handles batch c//2, heads [6*(c%2), 6*(c%2)+6).

Per-core algorithm (per head):
  QT/KT = (x @ W.T + b).T computed directly in [d, l] layout via host-pre-
  transposed x.T / W.T operands (bf16 matmuls, fp32 psum).
  Scores are built TRANSPOSED, S.T[r, l], so softmax'd probs come out in the
  [r-part, l-free] layout the P@V matmul needs (no P transpose):
    S1.T   : K-major qk matmul (swap lhsT/rhs).
    q*pos  : Dq_f[l, j'] = q[l] . E_flipped[j'] matmul -> DRAM scratch; the
             Toeplitz skew A[l, r] = Dq_f[l, 1023-l+r] is a pure RESHAPE of
             the flat scratch with row stride 1151 (contiguous DMA); blocks
             are then PE-transposed into S.T.
    k*pos  : Dk[r, j] = k[r] . E[j] -> scratch; B.T[r, l] = Dk[r, 1023+l-r]
             is the same stride-1151 reshape, already in [r, l] layout.
  Softmax skips max-subtraction (|scores/8| < ~3) and normalizes after PV:
  V is augmented with a ones-column so Z[l] falls out of the PV matmul.
"""

import sys

sys.path.insert(0, "/opt/trn_rl_repo")

from contextlib import ExitStack

import ml_dtypes
import numpy as np

import concourse.bass as bass
import concourse.tile as tile
from concourse import bacc, mybir
from concourse.bass_utils import run_bass_kernel_spmd
from concourse.masks import make_identity

BF16 = ml_dtypes.bfloat16
B, L, H, NH, HD = 4, 1024, 768, 12, 64
MAX_POS = 1024
NCORES = 8
HPC = NH // 2          # heads per core = 6
JPAD = 2176            # padded relative-index axis (>= 2047, mult of 128)
W = 1152               # per-tile j-window width (>= 1151, = 512+512+128)
ROWB = 128 * W         # flat scratch elements per l/r tile

_nc_cache = {}


def _build_nc():
    nc = bacc.Bacc(
        "TRN2",
        target_bir_lowering=False,
        debug=False,
        enable_asserts=True,
        num_devices=NCORES,
    )
    f32 = mybir.dt.float32
    bf16 = mybir.dt.bfloat16

    xfT = nc.dram_tensor("xfT", [H, L], bf16, kind="ExternalInput")
    xtT = nc.dram_tensor("xtT", [H, L], bf16, kind="ExternalInput")
    wqT = nc.dram_tensor("wqT", [H, HPC * HD], bf16, kind="ExternalInput")
    wkT = nc.dram_tensor("wkT", [H, HPC * HD], bf16, kind="ExternalInput")
    wvT = nc.dram_tensor("wvT", [H, HPC * HD], bf16, kind="ExternalInput")
    bqp = nc.dram_tensor("bqp", [128, 3], f32, kind="ExternalInput")
    bkp = nc.dram_tensor("bkp", [128, 3], f32, kind="ExternalInput")
    bvr = nc.dram_tensor("bvr", [1, HPC * HD], f32, kind="ExternalInput")
    ETd = nc.dram_tensor("ETd", [128, JPAD], bf16, kind="ExternalInput")
    EFTd = nc.dram_tensor("EFTd", [128, JPAD], bf16, kind="ExternalInput")
    out = nc.dram_tensor("out", [L, HPC * HD], f32, kind="ExternalOutput")

    Ident = mybir.ActivationFunctionType.Identity
    Exp = mybir.ActivationFunctionType.Exp
    add = mybir.AluOpType.add
    mult = mybir.AluOpType.mult

    with tile.TileContext(nc) as tc, ExitStack() as ctx:
        const = ctx.enter_context(tc.tile_pool(name="const", bufs=1))
        stg_pool = ctx.enter_context(tc.tile_pool(name="stg", bufs=4))
        bt_pool = ctx.enter_context(tc.tile_pool(name="btp", bufs=3))
        s_pool = ctx.enter_context(tc.tile_pool(name="sp", bufs=2))
        ept_pool = ctx.enter_context(tc.tile_pool(name="eptp", bufs=3))
        ctxt_pool = ctx.enter_context(tc.tile_pool(name="ctxtp", bufs=2))
        zr_pool = ctx.enter_context(tc.tile_pool(name="zrp", bufs=4))
        pp_pool = ctx.enter_context(tc.tile_pool(name="ppp", bufs=1, space="PSUM"))
        pst_pool = ctx.enter_context(tc.tile_pool(name="pstp", bufs=1, space="PSUM"))
        pat_pool = ctx.enter_context(tc.tile_pool(name="patp", bufs=1, space="PSUM"))
        pct_pool = ctx.enter_context(tc.tile_pool(name="pctp", bufs=1, space="PSUM"))
        dram_pool = ctx.enter_context(tc.tile_pool(name="scr", bufs=2, space="DRAM"))

        # ---- constant loads ----
        xf_sb = const.tile([128, 6, L], bf16)
        nc.sync.dma_start(xf_sb, xfT.ap().rearrange("(t p) l -> p t l", p=128))
        xt_sb = const.tile([128, 6, L], bf16)
        nc.sync.dma_start(xt_sb, xtT.ap().rearrange("(t p) l -> p t l", p=128))
        wq_sb = const.tile([128, 6, HPC * HD], bf16)
        nc.sync.dma_start(wq_sb, wqT.ap().rearrange("(t p) d -> p t d", p=128))
        wk_sb = const.tile([128, 6, HPC * HD], bf16)
        nc.sync.dma_start(wk_sb, wkT.ap().rearrange("(t p) d -> p t d", p=128))
        wv_sb = const.tile([128, 6, HPC * HD], bf16)
        nc.sync.dma_start(wv_sb, wvT.ap().rearrange("(t p) d -> p t d", p=128))
        bq_sb = const.tile([128, 3], f32)
        nc.sync.dma_start(bq_sb, bqp.ap())
        bk_sb = const.tile([128, 3], f32)
        nc.sync.dma_start(bk_sb, bkp.ap())
        et_sb = const.tile([128, JPAD], bf16)
        nc.sync.dma_start(et_sb, ETd.ap())
        eft_sb = const.tile([128, JPAD], bf16)
        nc.sync.dma_start(eft_sb, EFTd.ap())
        bv_bc = const.tile([128, HPC * HD], f32)
        bv_ap = bvr.ap()
        nc.gpsimd.dma_start(
            bv_bc,
            bass.AP(tensor=bv_ap.tensor, offset=bv_ap.offset,
                    ap=[[0, 128]] + bv_ap.ap[1:]),
        )
        ident_bf = const.tile([128, 128], bf16)
        make_identity(nc, ident_bf)
        ident_f32 = const.tile([128, 128], f32)
        make_identity(nc, ident_f32)

        qt_sb = const.tile([128, 3, L], bf16)
        kt_sb = const.tile([128, 3, L], bf16)
        vaug_sb = const.tile([128, 8, HPC * 65], bf16)
        out_sb = const.tile([128, 8, HPC * HD], f32)
        a_sb = const.tile([128, 8, L], bf16)

        # ---- projections ----
        for w_sb, b_sb, dst in ((wq_sb, bq_sb, qt_sb), (wk_sb, bk_sb, kt_sb)):
            for dt in range(3):
                for nh in range(2):
                    ps = pp_pool.tile([128, 1536], f32, tag="pbig")
                    for ki in range(6):
                        nc.tensor.matmul(
                            ps[:, 0:512],
                            lhsT=w_sb[:, ki, dt * 128:(dt + 1) * 128],
                            rhs=xf_sb[:, ki, nh * 512:(nh + 1) * 512]
                            if dst is qt_sb
                            else xt_sb[:, ki, nh * 512:(nh + 1) * 512],
                            start=(ki == 0),
                            stop=(ki == 5),
                        )
                    nc.scalar.activation(
                        out=dst[:, dt, nh * 512:(nh + 1) * 512],
                        in_=ps[:, 0:512],
                        func=Ident,
                        bias=b_sb[:, dt:dt + 1],
                        scale=1.0,
                    )
        for rt in range(8):
            ps = pp_pool.tile([128, 1536], f32, tag="pbig")
            for ki in range(6):
                nc.tensor.matmul(
                    ps[:, 0:HPC * HD],
                    lhsT=xt_sb[:, ki, rt * 128:(rt + 1) * 128],
                    rhs=wv_sb[:, ki, :],
                    start=(ki == 0),
                    stop=(ki == 5),
                )
            nc.vector.tensor_tensor(
                vaug_sb[:, rt].rearrange("p (h e) -> p h e", e=65)[:, :, 0:HD],
                ps[:, 0:HPC * HD].rearrange("p (h d) -> p h d", d=HD),
                bv_bc.rearrange("p (h d) -> p h d", d=HD),
                add,
            )
        nc.vector.memset(
            vaug_sb.rearrange("p r (h e) -> p r h e", e=65)[:, :, :, 64:65], 1.0
        )

        # ---- per-head attention ----
        copyflip = 0
        for h in range(HPC):
            hp = h // 2
            RH = slice(64 * (h % 2), 64 * (h % 2) + 64)
            dqscr = dram_pool.tile([8 * ROWB], bf16, tag="dqscr")
            dkscr = dram_pool.tile([8 * ROWB], bf16, tag="dkscr")

            # scratch matmuls: Dq_f (flipped E) per l-tile, Dk (plain E) per r-tile
            for lt in range(8):
                w0 = 896 - lt * 128
                for src_sb, scr, qk in ((eft_sb, dqscr, qt_sb), (et_sb, dkscr, kt_sb)):
                    ps = pp_pool.tile([128, 1536], f32, tag="pbig")
                    for c, cw in ((0, 512), (512, 512), (1024, 128)):
                        nc.tensor.matmul(
                            ps[:, c:c + cw],
                            lhsT=qk[RH, hp, lt * 128:(lt + 1) * 128],
                            rhs=src_sb[RH, w0 + c:w0 + c + cw],
                            start=True,
                            stop=True,
                        )
                    stg = stg_pool.tile([128, W], bf16, tag="stg")
                    if copyflip % 2 == 0:
                        nc.scalar.copy(stg, ps[:, 0:W])
                    else:
                        nc.vector.tensor_copy(stg, ps[:, 0:W])
                    copyflip += 1
                    nc.sync.dma_start(
                        scr[lt * ROWB:(lt + 1) * ROWB].rearrange("(p w) -> p w", w=W),
                        stg,
                    )
            # skew-reads of the q-side scratch (A[l, r], contiguous via stride-1151 view)
            for lt in range(8):
                base = lt * ROWB + 127
                nc.sync.dma_start(
                    a_sb[:, lt, :],
                    dqscr[base:base + 128 * (W - 1)]
                    .rearrange("(p w) -> p w", w=W - 1)[:, 0:L],
                )

            pct = pct_pool.tile([128, 1024], f32, tag="pct")
            for rt in range(8):
                pst = pst_pool.tile([128, 1024], f32, tag="pst")
                for nh in range(2):
                    nc.tensor.matmul(
                        pst[:, nh * 512:(nh + 1) * 512],
                        lhsT=kt_sb[RH, hp, rt * 128:(rt + 1) * 128],
                        rhs=qt_sb[RH, hp, nh * 512:(nh + 1) * 512],
                        start=True,
                        stop=True,
                    )
                pat = pat_pool.tile([128, 1024], bf16, tag="pat")
                for lt in range(8):
                    nc.tensor.matmul(
                        pat[:, lt * 128:(lt + 1) * 128],
                        lhsT=a_sb[:, lt, rt * 128:(rt + 1) * 128],
                        rhs=ident_bf,
                        is_transpose=True,
                    )
                bt = bt_pool.tile([128, 1024], bf16, tag="bt")
                base = rt * ROWB + 127
                nc.sync.dma_start(
                    bt,
                    dkscr[base:base + 128 * (W - 1)]
                    .rearrange("(p w) -> p w", w=W - 1)[:, 0:L],
                )
                s1 = s_pool.tile([128, 1024], f32, tag="s1")
                nc.vector.tensor_tensor(s1, pst, bt, add)
                s2 = s_pool.tile([128, 1024], f32, tag="s2")
                nc.vector.tensor_tensor(s2, s1, pat, add)
                ept = ept_pool.tile([128, 1024], bf16, tag="ept")
                nc.scalar.activation(ept, s2, Exp, scale=0.125)
                for nh in range(2):
                    nc.tensor.matmul(
                        pct[0:65, nh * 512:(nh + 1) * 512],
                        lhsT=vaug_sb[:, rt, h * 65:h * 65 + 65],
                        rhs=ept[:, nh * 512:(nh + 1) * 512],
                        start=(rt == 0),
                        stop=(rt == 7),
                    )
            ctxt = ctxt_pool.tile([128, 1024], f32, tag="ctxt")
            nc.scalar.copy(ctxt[0:65, :], pct[0:65, :])
            for lt in range(8):
                ctr = pat_pool.tile([128, 128], f32, tag="pat")
                nc.tensor.matmul(
                    ctr[:, 0:65],
                    lhsT=ctxt[0:65, lt * 128:(lt + 1) * 128],
                    rhs=ident_f32[0:65, 0:65],
                    is_transpose=True,
                )
                zr = zr_pool.tile([128, 1], f32, tag="zr")
                nc.vector.reciprocal(zr, ctr[:, 64:65])
                nc.vector.tensor_tensor(
                    out_sb[:, lt, h * HD:(h + 1) * HD],
                    ctr[:, 0:HD],
                    zr.to_broadcast([128, HD]),
                    mult,
                )
        for lt in range(8):
            nc.sync.dma_start(out.ap()[lt * 128:(lt + 1) * 128, :], out_sb[:, lt, :])

    nc.compile()
    return nc


def get_nc():
    if "nc" not in _nc_cache:
        _nc_cache["nc"] = _build_nc()
    return _nc_cache["nc"]


def make_in_maps(from_tensor, to_tensor, Wq, bq, Wk, bk, Wv, bv, dist_emb):
    E = np.asarray(dist_emb, np.float32)
    Epad = np.zeros((JPAD, HD), np.float32)
    Epad[: 2 * MAX_POS - 1] = E
    EFpad = np.zeros((JPAD, HD), np.float32)
    EFpad[: 2 * MAX_POS - 1] = E[::-1]
    ETd = np.ascontiguousarray(
        np.vstack([Epad.T, Epad.T]).astype(BF16)
    )
    EFTd = np.ascontiguousarray(np.vstack([EFpad.T, EFpad.T]).astype(BF16))

    in_maps = []
    for c in range(NCORES):
        b = c // 2
        h0 = (c % 2) * HPC
        sl = slice(h0 * HD, (h0 + HPC) * HD)
        in_maps.append(
            {
                "xfT": np.ascontiguousarray(np.asarray(from_tensor[b], np.float32).T).astype(BF16),
                "xtT": np.ascontiguousarray(np.asarray(to_tensor[b], np.float32).T).astype(BF16),
                "wqT": np.ascontiguousarray(np.asarray(Wq, np.float32)[sl].T).astype(BF16),
                "wkT": np.ascontiguousarray(np.asarray(Wk, np.float32)[sl].T).astype(BF16),
                "wvT": np.ascontiguousarray(np.asarray(Wv, np.float32)[sl].T).astype(BF16),
                "bqp": np.ascontiguousarray(np.asarray(bq, np.float32)[sl].reshape(3, 128).T),
                "bkp": np.ascontiguousarray(np.asarray(bk, np.float32)[sl].reshape(3, 128).T),
                "bvr": np.asarray(bv, np.float32)[sl].reshape(1, HPC * HD).copy(),
                "ETd": ETd,
                "EFTd": EFTd,
            }
        )
    return in_maps


def assemble(results):
    full = np.zeros((B, L, H), np.float32)
    for c in range(NCORES):
        b = c // 2
        h0 = (c % 2) * HPC
        full[b, :, h0 * HD:(h0 + HPC) * HD] = results[c]["out"]
    return full


def kernel(**inputs):
    in_maps = make_in_maps(**inputs)
    nc = get_nc()
    res = run_bass_kernel_spmd(nc, in_maps, core_ids=list(range(NCORES)))
    return assemble(res.results)


if __name__ == "__main__":
    rng = np.random.default_rng(0)
    ins = {
        "from_tensor": rng.standard_normal((B, L, H), dtype=np.float32),
        "to_tensor": rng.standard_normal((B, L, H), dtype=np.float32),
        "Wq": rng.standard_normal((H, H), dtype=np.float32) * 0.02,
        "bq": rng.standard_normal((H,), dtype=np.float32) * 0.02,
        "Wk": rng.standard_normal((H, H), dtype=np.float32) * 0.02,
        "bk": rng.standard_normal((H,), dtype=np.float32) * 0.02,
        "Wv": rng.standard_normal((H, H), dtype=np.float32) * 0.02,
        "bv": rng.standard_normal((H,), dtype=np.float32) * 0.02,
        "dist_emb": rng.standard_normal((2 * MAX_POS - 1, HD), dtype=np.float32) * 0.02,
    }
    out = kernel(**ins)
    print("ran", out.shape, out.dtype)
